# revision 20
# baseline (speedup 1.0000x reference)
"""Trainium2 Bass kernel for the CombineGraph GNN forward pass.

Strategy: data-parallel over batch across 8 NeuronCores (replicate the
embedding table + small weights, shard inputs/adj/last_items on B).

Per core (64 sessions): tokens are laid out in "session-pair" tiles of
100 partitions (2 sessions x 50 positions).  All embedding lookups are
done with indirect DMA gathers whose offset tiles are int32 columns in
SBUF; the [P, k] offset layout gathers k rows per partition into a
[P, k*D] tile in a single instruction.

Self-contained: hardcodes all shapes from the problem spec.
"""

import numpy as np

from concourse import bass, mybir
from concourse.bass import IndirectOffsetOnAxis
from concourse.tile import TileContext
from concourse.bass_utils import run_bass_kernel_spmd

# ---- problem dims (hardcoded) ----
N_CORES = 8
N_NODE = 500000
D = 100
B = 512
S = 50
NS = 12
LAST = 3
NADJ = 12
SCALE = float(1.0 / np.sqrt(D))

BC = B // N_CORES        # 64 sessions per core
NPAIR = BC // 2          # 32 session pairs per core
G = 4                    # session pairs per chunk
NCHUNK = NPAIR // G      # 8 chunks
PT = 2 * S               # 100 partitions per pair tile
HBC = BC // 2            # 32 sessions per "last" half
LP = HBC * LAST          # 96 partitions for last-items branch

f32 = mybir.dt.float32
i32 = mybir.dt.int32
AF = mybir.ActivationFunctionType
OP = mybir.AluOpType


def build_program() -> bass.Bass:
    nc = bass.Bass()

    # ---- DRAM I/O ----
    emb = nc.dram_tensor("emb", [N_NODE, D], f32, kind="ExternalInput")
    adj_all = nc.dram_tensor("adj_all", [N_NODE, NS], i32, kind="ExternalInput")
    num_w = nc.dram_tensor("num_w", [N_NODE, NS], f32, kind="ExternalInput")

    idx_cols = nc.dram_tensor("idx_cols", [PT, NPAIR], i32, kind="ExternalInput")
    adjT = nc.dram_tensor("adjT", [BC, S, S], f32, kind="ExternalInput")
    last_cols = nc.dram_tensor("last_cols", [LP, 2], i32, kind="ExternalInput")
    adjit_cols = nc.dram_tensor("adjit_cols", [LP, 2 * NADJ], i32, kind="ExternalInput")

    w_a = nc.dram_tensor("w_a", [D, D], f32, kind="ExternalInput")
    w_k = nc.dram_tensor("w_k", [D, D], f32, kind="ExternalInput")
    w_qT = nc.dram_tensor("w_qT", [D, D], f32, kind="ExternalInput")
    w_gh = nc.dram_tensor("w_gh", [D, D], f32, kind="ExternalInput")
    w_ga = nc.dram_tensor("w_ga", [D, D], f32, kind="ExternalInput")
    w_l = nc.dram_tensor("w_l", [D, D], f32, kind="ExternalInput")
    s_cols = nc.dram_tensor("s_cols", [PT, 2], f32, kind="ExternalInput")
    ones2 = nc.dram_tensor("ones2", [PT, 2], f32, kind="ExternalInput")
    ones1 = nc.dram_tensor("ones1", [PT, 1], f32, kind="ExternalInput")
    onesr = nc.dram_tensor("onesr", [1, PT], f32, kind="ExternalInput")
    sel_all = nc.dram_tensor("sel_all", [BC, NPAIR * PT], f32, kind="ExternalInput")
    ident = nc.dram_tensor("ident", [128, 128], f32, kind="ExternalInput")
    third = nc.dram_tensor("third", [LP, HBC], f32, kind="ExternalInput")

    # output layout [two, s, pair, d]; host unpermutes to [64, 50, 100]
    outT = nc.dram_tensor("outT", [2, S, NPAIR, D], f32, kind="ExternalOutput")
    lasth = nc.dram_tensor("lasth", [2, HBC, D], f32, kind="ExternalOutput")

    adjT_r = adjT.rearrange("(x two) t s -> two t x s", two=2)

    with TileContext(nc) as tc:
        with (
            tc.tile_pool(name="mpool", bufs=1) as mp,
            tc.tile_pool(name="kpool", bufs=2) as kp,
            tc.tile_pool(name="npool", bufs=2) as np_,
            tc.tile_pool(name="ppool", bufs=3) as pp,
        ):
            # ---- persistent loads ----
            idx_sb = mp.tile([PT, NPAIR], i32)
            nc.sync.dma_start(out=idx_sb[:, :], in_=idx_cols[:, :])
            wa_sb = mp.tile([D, D], f32)
            nc.sync.dma_start(out=wa_sb[:, :], in_=w_a[:, :])
            wk_sb = mp.tile([D, D], f32)
            nc.sync.dma_start(out=wk_sb[:, :], in_=w_k[:, :])
            wqT_sb = mp.tile([D, D], f32)
            nc.sync.dma_start(out=wqT_sb[:, :], in_=w_qT[:, :])
            wgh_sb = mp.tile([D, D], f32)
            nc.sync.dma_start(out=wgh_sb[:, :], in_=w_gh[:, :])
            wga_sb = mp.tile([D, D], f32)
            nc.sync.dma_start(out=wga_sb[:, :], in_=w_ga[:, :])
            wl_sb = mp.tile([D, D], f32)
            nc.sync.dma_start(out=wl_sb[:, :], in_=w_l[:, :])
            scol_sb = mp.tile([PT, 2], f32)
            nc.sync.dma_start(out=scol_sb[:, :], in_=s_cols[:, :])
            ones2_sb = mp.tile([PT, 2], f32)
            nc.sync.dma_start(out=ones2_sb[:, :], in_=ones2[:, :])
            ones1_sb = mp.tile([PT, 1], f32)
            nc.sync.dma_start(out=ones1_sb[:, :], in_=ones1[:, :])
            onesr_sb = mp.tile([1, PT], f32)
            nc.sync.dma_start(out=onesr_sb[:, :], in_=onesr[:, :])
            sel_sb = mp.tile([BC, NPAIR * PT], f32)
            nc.sync.dma_start(out=sel_sb[:, :], in_=sel_all[:, :])
            ident_sb = mp.tile([128, 128], f32)
            nc.sync.dma_start(out=ident_sb[:, :], in_=ident[:, :])
            third_sb = mp.tile([LP, HBC], f32)
            nc.sync.dma_start(out=third_sb[:, :], in_=third[:, :])
            lastc_sb = mp.tile([LP, 2], i32)
            nc.sync.dma_start(out=lastc_sb[:, :], in_=last_cols[:, :])
            adjitc_sb = mp.tile([LP, 2 * NADJ], i32)
            nc.sync.dma_start(out=adjitc_sb[:, :], in_=adjit_cols[:, :])

            # ---- big index-driven gathers (issued up front) ----
            # split into <=800-descriptor instructions (8 offset columns):
            # larger single indirect DMAs wedge the SWDGE ring on HW
            h_all = mp.tile([PT, NPAIR * D], f32)
            for j in range(NPAIR):
                nc.gpsimd.indirect_dma_start(
                    out=h_all[:, j * D:(j + 1) * D], out_offset=None,
                    in_=emb[:, :],
                    in_offset=IndirectOffsetOnAxis(
                        ap=idx_sb[:, j:j + 1], axis=0),
                )
            nidx = mp.tile([PT, NPAIR * NS], i32)
            for j in range(NPAIR):
                nc.gpsimd.indirect_dma_start(
                    out=nidx[:, j * NS:(j + 1) * NS], out_offset=None,
                    in_=adj_all[:, :],
                    in_offset=IndirectOffsetOnAxis(
                        ap=idx_sb[:, j:j + 1], axis=0),
                )
            numw = mp.tile([PT, NPAIR * NS], f32)
            for j in range(NPAIR):
                nc.gpsimd.indirect_dma_start(
                    out=numw[:, j * NS:(j + 1) * NS], out_offset=None,
                    in_=num_w[:, :],
                    in_offset=IndirectOffsetOnAxis(
                        ap=idx_sb[:, j:j + 1], axis=0),
                )
            # last-items branch gathers
            item_sb = [None, None]
            adjh_sb = [None, None]
            for ch in range(2):
                item_sb[ch] = mp.tile([LP, D], f32, name=f"item{ch}")
                nc.gpsimd.indirect_dma_start(
                    out=item_sb[ch][:, :], out_offset=None, in_=emb[:, :],
                    in_offset=IndirectOffsetOnAxis(ap=lastc_sb[:, ch:ch + 1], axis=0),
                )
                adjh_sb[ch] = mp.tile([LP, NADJ * D], f32, name=f"adjh{ch}")
                for j in range(NADJ):
                    nc.gpsimd.indirect_dma_start(
                        out=adjh_sb[ch][:, j * D:(j + 1) * D], out_offset=None,
                        in_=emb[:, :],
                        in_offset=IndirectOffsetOnAxis(
                            ap=adjitc_sb[:, ch * NADJ + j:ch * NADJ + j + 1],
                            axis=0),
                    )

            # ---- mask ----
            mask_sb = mp.tile([PT, NPAIR], f32)
            nc.vector.tensor_scalar(
                out=mask_sb[:, :], in0=idx_sb[:, :], scalar1=0, scalar2=None,
                op0=OP.not_equal,
            )
            # mask2[p, 2i+ch] = mask[p, i] * (half(p) == ch)
            # (two 2D strided tensor_scalar ops: 3D TensorTensor encodings
            # have too few sync-wait slots for walrus)
            mask2 = mp.tile([PT, BC], f32)
            mask2_v = mask2[:, :].rearrange("p (i c) -> p c i", c=2)
            for ch in range(2):
                nc.vector.tensor_scalar_mul(
                    out=mask2_v[:, ch, :], in0=mask_sb[:, :],
                    scalar1=ones2_sb[:, ch:ch + 1])

            # =========================================================
            # PASS A: per-session masked mean, accumulated transposed:
            #   starT[d, b] = sum_t h[t, d] * winv2[t, b]
            # =========================================================
            with tc.tile_pool(name="psA", bufs=2, space="PSUM") as psA:
                cnt_ps = psA.tile([1, BC], f32, tag="cnt", bufs=1)
                nc.tensor.matmul(out=cnt_ps[:, :], lhsT=ones1_sb[:, :],
                                 rhs=mask2[:, :], start=True, stop=True)
                inv64 = mp.tile([1, BC], f32)
                nc.vector.tensor_scalar_max(out=inv64[:, :], in0=cnt_ps[:, :],
                                            scalar1=1.0)
                nc.vector.reciprocal(out=inv64[:, :], in_=inv64[:, :])
                # winv2 = mask2 * inv64 (per column); broadcast inv64 across
                # partitions with a k=1 matmul
                invb_ps = psA.tile([PT, BC], f32, tag="invb", bufs=1)
                nc.tensor.matmul(out=invb_ps[:, :], lhsT=onesr_sb[:, :],
                                 rhs=inv64[:, :], start=True, stop=True)
                winv2 = mp.tile([PT, BC], f32)
                nc.vector.tensor_tensor(
                    out=winv2[:, :], in0=mask2[:, :], in1=invb_ps[:, :],
                    op=OP.mult)

                starT_ps = psA.tile([D, BC], f32, tag="starT", bufs=1)
                for i in range(NPAIR):
                    nc.tensor.matmul(
                        out=starT_ps[:, 2 * i:2 * i + 2],
                        lhsT=h_all[:, i * D:(i + 1) * D],
                        rhs=winv2[:, 2 * i:2 * i + 2],
                        start=True, stop=True)
                starT = mp.tile([D, BC], f32)
                nc.scalar.copy(out=starT[:, :], in_=starT_ps[:, :])

                # ---- mid phase: star64, starWk, sq ----
                star64_ps = psA.tile([BC, D], f32, tag="mid")
                nc.tensor.transpose(out=star64_ps[:, :], in_=starT[:, :],
                                    identity=ident_sb[0:D, 0:D])
                ss64 = mp.tile([BC, 2 * D], f32)
                nc.vector.tensor_copy(out=ss64[:, 0:D], in_=star64_ps[:, :])
                swkT_ps = psA.tile([D, BC], f32, tag="mid")
                nc.tensor.matmul(out=swkT_ps[:, :], lhsT=wk_sb[:, :],
                                 rhs=starT[:, :], start=True, stop=True)
                swkT = mp.tile([D, BC], f32)
                nc.scalar.copy(out=swkT[:, :], in_=swkT_ps[:, :])
                sq64_ps = psA.tile([BC, D], f32, tag="mid")
                nc.tensor.matmul(out=sq64_ps[:, :], lhsT=swkT[:, :],
                                 rhs=wqT_sb[:, :], start=True, stop=True)
                nc.scalar.copy(out=ss64[:, D:2 * D], in_=sq64_ps[:, :])

            # =========================================================
            # PASS B: per-chunk global/local branches
            # =========================================================
            with tc.tile_pool(name="psB", bufs=1, space="PSUM") as psB:
                for c in range(NCHUNK):
                    # neighbor embedding gather for this chunk
                    neigh = np_.tile([PT, G * NS * D], f32, tag="neigh")
                    for j in range(G * NS):
                        nc.gpsimd.indirect_dma_start(
                            out=neigh[:, j * D:(j + 1) * D], out_offset=None,
                            in_=emb[:, :],
                            in_offset=IndirectOffsetOnAxis(
                                ap=nidx[:, c * G * NS + j:c * G * NS + j + 1],
                                axis=0),
                        )
                    # block-diagonal transposed adjacency for the chunk
                    adjt = kp.tile([PT, G * PT], f32, tag="adjt")
                    nc.vector.memset(adjt[:, :], 0.0)
                    adjt_v = adjt[:, :].rearrange("p (g q) -> p g q", g=G)
                    nc.sync.dma_start(
                        out=adjt_v[0:S, :, 0:S],
                        in_=adjT_r[0, :, c * G:(c + 1) * G, :])
                    nc.sync.dma_start(
                        out=adjt_v[S:PT, :, S:PT],
                        in_=adjT_r[1, :, c * G:(c + 1) * G, :])

                    # ---- softmax over neighbor weights (chunk level) ----
                    nw = numw[:, c * G * NS:(c + 1) * G * NS]
                    # 2D touches absorb cross-engine waits so the 3D ops
                    # below carry at most one sync wait (walrus limit)
                    tch = pp.tile([PT, 1], f32, tag="tch")
                    nc.vector.tensor_copy(out=tch[:, :], in_=nw[:, 0:1])
                    tch2 = pp.tile([PT, 1], f32, tag="tch2")
                    nc.scalar.copy(out=tch2[:, :], in_=nw[:, 0:1])
                    negmax = kp.tile([PT, G], f32, tag="negmax")
                    nc.vector.tensor_reduce(
                        out=negmax[:, :], in_=nw.rearrange("p (g j) -> p g j", g=G),
                        axis=mybir.AxisListType.X, op=OP.max, negate=True)
                    expw = kp.tile([PT, G * NS], f32, tag="expw")
                    sumexp = kp.tile([PT, G], f32, tag="sumexp")
                    for g in range(G):
                        nc.scalar.activation(
                            out=expw[:, g * NS:(g + 1) * NS],
                            in_=nw[:, g * NS:(g + 1) * NS],
                            func=AF.Exp, bias=negmax[:, g:g + 1], scale=1.0,
                            accum_out=sumexp[:, g:g + 1])
                    invse = kp.tile([PT, G], f32, tag="invse")
                    nc.vector.reciprocal(out=invse[:, :], in_=sumexp[:, :])
                    alpha = kp.tile([PT, G * NS], f32, tag="alpha")
                    for g in range(G):
                        nc.vector.tensor_scalar_mul(
                            out=alpha[:, g * NS:(g + 1) * NS],
                            in0=expw[:, g * NS:(g + 1) * NS],
                            scalar1=invse[:, g:g + 1])
                    # weighted neighbors (out of place: in-place DVE with a
                    # broadcast in1 corrupts data on HW) then reduce over j
                    tch3 = pp.tile([PT, 1], f32, tag="tch3")
                    nc.vector.tensor_copy(out=tch3[:, :], in_=neigh[:, 0:1])
                    wtd = np_.tile([PT, G * NS * D], f32, tag="wtd")
                    nc.vector.tensor_tensor(
                        out=wtd[:, :].rearrange("p (n d) -> p n d", n=G * NS),
                        in0=neigh[:, :].rearrange("p (n d) -> p n d", n=G * NS),
                        in1=alpha[:, :].rearrange("p (n o) -> p n o", o=1
                                                  ).broadcast_to((PT, G * NS, D)),
                        op=OP.mult)
                    agg = kp.tile([PT, G * D], f32, tag="agg")
                    nc.vector.tensor_reduce(
                        out=agg[:, :].rearrange("p (g d) -> p g d", g=G),
                        in_=wtd[:, :].rearrange("p (g j d) -> p g d j", g=G, j=NS),
                        axis=mybir.AxisListType.X, op=OP.add)

                    out_chunk = kp.tile([PT, G * D], f32, tag="out_chunk")

                    for g in range(G):
                        i = c * G + g
                        hsl = h_all[:, i * D:(i + 1) * D]
                        # star/sq broadcast to token partitions
                        bc_ps = psB.tile([PT, 2 * D], f32, tag="bc")
                        nc.tensor.matmul(out=bc_ps[:, :],
                                         lhsT=sel_sb[:, i * PT:(i + 1) * PT],
                                         rhs=ss64[:, :], start=True, stop=True)
                        # hT
                        hT_ps = psB.tile([D, PT], f32, tag="hT")
                        nc.tensor.transpose(out=hT_ps[:, :], in_=hsl,
                                            identity=ident_sb[0:PT, 0:PT])
                        hT = pp.tile([D, PT], f32, tag="hT_sb")
                        nc.scalar.copy(out=hT[:, :], in_=hT_ps[:, :])
                        # h @ W_star_a
                        hwa_ps = psB.tile([PT, D], f32, tag="hwa")
                        nc.tensor.matmul(out=hwa_ps[:, :], lhsT=hT[:, :],
                                         rhs=wa_sb[:, :], start=True, stop=True)
                        hwa = pp.tile([PT, D], f32, tag="hwa_sb")
                        nc.vector.tensor_copy(out=hwa[:, :], in_=hwa_ps[:, :])
                        # h_n = adj @ (h @ W_a), block-diagonal over the 2 sessions
                        hn_ps = psB.tile([PT, D], f32, tag="hn")
                        nc.tensor.matmul(out=hn_ps[:, :],
                                         lhsT=adjt[:, g * PT:(g + 1) * PT],
                                         rhs=hwa[:, :], start=True, stop=True)
                        # gate = sigmoid(SCALE * <h, sq_b>)
                        tt_scr = pp.tile([PT, D], f32, tag="tt_scr")
                        gate_pre = pp.tile([PT, 1], f32, tag="gate_pre")
                        nc.vector.tensor_tensor(out=tt_scr[:, :], in0=hsl,
                                                in1=bc_ps[:, D:2 * D], op=OP.mult)
                        nc.vector.tensor_reduce(
                            out=gate_pre[:, :], in_=tt_scr[:, :],
                            axis=mybir.AxisListType.X, op=OP.add)
                        gate = pp.tile([PT, 1], f32, tag="gate")
                        nc.scalar.activation(out=gate[:, :], in_=gate_pre[:, :],
                                             func=AF.Sigmoid, scale=SCALE)
                        og = pp.tile([PT, 1], f32, tag="og")
                        nc.vector.tensor_scalar(
                            out=og[:, :], in0=gate[:, :], scalar1=-1.0, scalar2=1.0,
                            op0=OP.mult, op1=OP.add)
                        # hl = og * h_n + gate * star_b
                        t2 = pp.tile([PT, D], f32, tag="t2")
                        nc.scalar.mul(out=t2[:, :], in_=bc_ps[:, 0:D], mul=gate[:, :1])
                        hl = pp.tile([PT, D], f32, tag="hl")
                        nc.vector.tensor_scalar_mul(out=hl[:, :], in0=hn_ps[:, :],
                                                    scalar1=og[:, :1])
                        nc.vector.tensor_add(out=hl[:, :], in0=hl[:, :], in1=t2[:, :])
                        # local l2norm, scaled by s0 -> out_chunk
                        sq_scr = pp.tile([PT, D], f32, tag="sq_scr")
                        ssum = pp.tile([PT, 1], f32, tag="ssum")
                        nc.scalar.activation(out=sq_scr[:, :], in_=hl[:, :],
                                             func=AF.Square, accum_out=ssum[:, :])
                        nrm = pp.tile([PT, 1], f32, tag="nrm")
                        nc.scalar.sqrt(out=nrm[:, :], in_=ssum[:, :])
                        nc.vector.tensor_scalar_max(out=nrm[:, :], in0=nrm[:, :],
                                                    scalar1=1e-12)
                        invl = pp.tile([PT, 1], f32, tag="invl")
                        nc.vector.reciprocal(out=invl[:, :], in_=nrm[:, :])
                        osl = out_chunk[:, g * D:(g + 1) * D]
                        nc.vector.tensor_scalar(
                            out=osl, in0=hl[:, :], scalar1=invl[:, :1],
                            scalar2=scol_sb[:, 0:1], op0=OP.mult, op1=OP.mult)

                        # ---- global branch ----
                        aggT_ps = psB.tile([D, PT], f32, tag="aggT")
                        nc.tensor.transpose(out=aggT_ps[:, :],
                                            in_=agg[:, g * D:(g + 1) * D],
                                            identity=ident_sb[0:PT, 0:PT])
                        aggT = pp.tile([D, PT], f32, tag="aggT_sb")
                        nc.scalar.copy(out=aggT[:, :], in_=aggT_ps[:, :])
                        hg_ps = psB.tile([PT, D], f32, tag="hg")
                        nc.tensor.matmul(out=hg_ps[:, :], lhsT=hT[:, :],
                                         rhs=wgh_sb[:, :], start=True, stop=False)
                        nc.tensor.matmul(out=hg_ps[:, :], lhsT=aggT[:, :],
                                         rhs=wga_sb[:, :], start=False, stop=True)
                        hg = pp.tile([PT, D], f32, tag="hg_sb")
                        nc.scalar.activation(out=hg[:, :], in_=hg_ps[:, :],
                                             func=AF.Relu)
                        sq_scr2 = pp.tile([PT, D], f32, tag="sq_scr2")
                        ssg = pp.tile([PT, 1], f32, tag="ssg")
                        nc.scalar.activation(out=sq_scr2[:, :], in_=hg[:, :],
                                             func=AF.Square, accum_out=ssg[:, :])
                        nrmg = pp.tile([PT, 1], f32, tag="nrmg")
                        nc.scalar.sqrt(out=nrmg[:, :], in_=ssg[:, :])
                        nc.vector.tensor_scalar_max(out=nrmg[:, :], in0=nrmg[:, :],
                                                    scalar1=1e-12)
                        invg = pp.tile([PT, 1], f32, tag="invg")
                        nc.vector.reciprocal(out=invg[:, :], in_=nrmg[:, :])
                        tmp_g = pp.tile([PT, D], f32, tag="tmp_g")
                        nc.vector.tensor_scalar(
                            out=tmp_g[:, :], in0=hg[:, :], scalar1=invg[:, :1],
                            scalar2=scol_sb[:, 1:2], op0=OP.mult, op1=OP.mult)
                        nc.vector.tensor_add(out=osl, in0=osl, in1=tmp_g[:, :])

                    # write chunk output
                    for ch in range(2):
                        nc.sync.dma_start(
                            out=outT[ch, :, c * G:(c + 1) * G, :],
                            in_=out_chunk[ch * S:(ch + 1) * S, :].rearrange(
                                "p (g d) -> p g d", g=G),
                        )

                # =====================================================
                # Last-items attention branch (PSUM tags reuse psB slots)
                # =====================================================
                for ch in range(2):
                    item = item_sb[ch]
                    adjh = adjh_sb[ch]
                    itemT_ps = psB.tile([D, LP], f32, tag="hT")
                    nc.tensor.transpose(out=itemT_ps[:, :], in_=item[:, :],
                                        identity=ident_sb[0:LP, 0:LP])
                    itemT = pp.tile([D, LP], f32, tag="itemT_sb")
                    nc.scalar.copy(out=itemT[:, :], in_=itemT_ps[:, :])
                    q_ps = psB.tile([LP, D], f32, tag="hwa")
                    nc.tensor.matmul(out=q_ps[:, :], lhsT=itemT[:, :],
                                     rhs=wl_sb[:, :], start=True, stop=True)
                    q = pp.tile([LP, D], f32, tag="q_sb")
                    nc.vector.tensor_copy(out=q[:, :], in_=q_ps[:, :])
                    # att logits: <q, adj_h_j>
                    tchl = pp.tile([LP, 1], f32, tag="tchl")
                    nc.vector.tensor_copy(out=tchl[:, :], in_=adjh[:, 0:1])
                    attscr = kp.tile([LP, NADJ * D], f32, tag="attscr")
                    nc.vector.tensor_tensor(
                        out=attscr[:, :].rearrange("p (j d) -> p j d", j=NADJ),
                        in0=adjh[:, :].rearrange("p (j d) -> p j d", j=NADJ),
                        in1=q[:, :].rearrange("p (o d) -> p o d", o=1
                                              ).broadcast_to((LP, NADJ, D)),
                        op=OP.mult)
                    attp = pp.tile([LP, NADJ], f32, tag="attp")
                    nc.vector.tensor_reduce(
                        out=attp[:, :],
                        in_=attscr[:, :].rearrange("p (j d) -> p j d", j=NADJ),
                        axis=mybir.AxisListType.X, op=OP.add)
                    # softmax over NADJ of SCALE*attp
                    nmx = pp.tile([LP, 1], f32, tag="nmx")
                    nc.vector.tensor_reduce(out=nmx[:, :], in_=attp[:, :],
                                            axis=mybir.AxisListType.X, op=OP.max,
                                            negate=True)
                    nc.vector.tensor_scalar_mul(out=nmx[:, :], in0=nmx[:, :],
                                                scalar1=SCALE)
                    expat = pp.tile([LP, NADJ], f32, tag="expat")
                    sexp = pp.tile([LP, 1], f32, tag="sexp")
                    nc.scalar.activation(out=expat[:, :], in_=attp[:, :], func=AF.Exp,
                                         bias=nmx[:, :1], scale=SCALE,
                                         accum_out=sexp[:, :])
                    isexp = pp.tile([LP, 1], f32, tag="isexp")
                    nc.vector.reciprocal(out=isexp[:, :], in_=sexp[:, :])
                    att = pp.tile([LP, NADJ], f32, tag="att")
                    nc.vector.tensor_scalar_mul(out=att[:, :], in0=expat[:, :],
                                                scalar1=isexp[:, :1])
                    # weighted sum of adj_h (out of place)
                    wadj = kp.tile([LP, NADJ * D], f32, tag="wadj")
                    nc.vector.tensor_tensor(
                        out=wadj[:, :].rearrange("p (j d) -> p j d", j=NADJ),
                        in0=adjh[:, :].rearrange("p (j d) -> p j d", j=NADJ),
                        in1=att[:, :].rearrange("p (j o) -> p j o", o=1
                                                ).broadcast_to((LP, NADJ, D)),
                        op=OP.mult)
                    aggl = pp.tile([LP, D], f32, tag="aggl")
                    nc.vector.tensor_reduce(
                        out=aggl[:, :],
                        in_=wadj[:, :].rearrange("p (j d) -> p d j", j=NADJ),
                        axis=mybir.AxisListType.X, op=OP.add)
                    nc.vector.tensor_add(out=aggl[:, :], in0=aggl[:, :],
                                         in1=item[:, :])
                    lh_ps = psB.tile([HBC, D], f32, tag="hn")
                    nc.tensor.matmul(out=lh_ps[:, :], lhsT=third_sb[:, :],
                                     rhs=aggl[:, :], start=True, stop=True)
                    lh = pp.tile([HBC, D], f32, tag="lh")
                    nc.vector.tensor_copy(out=lh[:, :], in_=lh_ps[:, :])
                    nc.sync.dma_start(out=lasth[ch, :, :], in_=lh[:, :])

    return nc


# ------------------------------------------------------------------
# sync-wait legalization: this walrus build accepts only ONE sync wait
# per engine instruction, but Tile emits several.  Move extra waits onto
# engine NOPs inserted immediately before the instruction (engines are
# in-order, so semantics are preserved).
# ------------------------------------------------------------------
_wait_carrier_n = [0]


def _make_wait_carrier(engine, wait):
    _wait_carrier_n[0] += 1
    inst = mybir.InstEventSemaphore(name=f"W-split-{_wait_carrier_n[0]}",
                                    ins=[], outs=[])
    inst.engine = engine
    inst.sync_info = mybir.SyncInfo(on_wait=[wait], on_update=[])
    return inst


def _make_semwrite_carrier(engine, sem_id):
    _wait_carrier_n[0] += 1
    inst = mybir.InstEventSemaphore(name=f"W-clr-{_wait_carrier_n[0]}",
                                    ins=[], outs=[])
    inst.engine = engine
    upd = mybir.SyncUpdate(sync_type="semaphore", id=sem_id,
                           ant_name=f"clr{sem_id}", update_mode="sem-wr-imm",
                           update_value=0, update_reg=None)
    inst.sync_info = mybir.SyncInfo(on_wait=[], on_update=[upd])
    return inst


def _split_multi_waits(nc):
    import re
    for fn in nc.m.functions:
        for bb in fn.blocks:
            insts = bb.instructions          # live list
            k = 0
            while k < len(insts):
                inst = insts[k]
                tname = type(inst).__name__
                if tname == "InstISA" and "SEMAPHORE_RANGE_CLEAR" in inst.concise():
                    # walrus rejects bass's encoding of this ISA op; replace
                    # with per-semaphore write-0 updates on event carriers
                    m = re.search(r"range_first=(\d+) range_last=(\d+)",
                                  inst.concise())
                    first, last = int(m.group(1)), int(m.group(2))
                    carriers = [_make_semwrite_carrier(inst.engine, sid)
                                for sid in range(first, last + 1)]
                    si = inst.sync_info
                    if si is not None and si.on_wait:
                        carriers[0].sync_info = mybir.SyncInfo(
                            on_wait=list(si.on_wait),
                            on_update=list(carriers[0].sync_info.on_update))
                    insts[k:k + 1] = carriers
                    k += len(carriers)
                    continue
                si = inst.sync_info
                if si is not None and len(si.on_wait) > 1:
                    waits = list(si.on_wait)
                    carriers = [_make_wait_carrier(inst.engine, w)
                                for w in waits[:-1]]
                    inst.sync_info = mybir.SyncInfo(
                        on_wait=[waits[-1]], on_update=list(si.on_update))
                    for j, cinst in enumerate(carriers):
                        insts.insert(k + j, cinst)
                    k += len(carriers)
                k += 1
    return nc


_PROGRAM = None
_PROGRAM_RAW = None


def _get_program(legalize=True):
    global _PROGRAM, _PROGRAM_RAW
    if legalize:
        if _PROGRAM is None:
            _PROGRAM = _split_multi_waits(build_program())
        return _PROGRAM
    if _PROGRAM_RAW is None:
        _PROGRAM_RAW = build_program()
    return _PROGRAM_RAW


def _prep_maps(inputs, adj, last_items, adj_items, embedding, s,
               W_star_a, W_star_q, W_star_k, W_g, W_l, adj_all, num_w):
    fi32 = lambda x: np.ascontiguousarray(np.asarray(x), dtype=np.int32)
    ff32 = lambda x: np.ascontiguousarray(np.asarray(x), dtype=np.float32)

    emb = ff32(embedding)
    adj_all32 = fi32(adj_all)
    num_w32 = ff32(num_w)
    shared = {
        "emb": emb,
        "adj_all": adj_all32,
        "num_w": num_w32,
        "w_a": ff32(W_star_a),
        "w_k": ff32(W_star_k),
        "w_qT": ff32(np.asarray(W_star_q).T),
        "w_gh": ff32(np.asarray(W_g)[:D]),
        "w_ga": ff32(np.asarray(W_g)[D:]),
        "w_l": ff32(W_l),
        "s_cols": ff32(np.tile(np.asarray(s).reshape(1, 2), (PT, 1))),
        "ident": np.eye(128, dtype=np.float32),
    }
    ones2 = np.zeros((PT, 2), np.float32)
    ones2[:S, 0] = 1.0
    ones2[S:, 1] = 1.0
    shared["ones2"] = ones2
    shared["ones1"] = np.ones((PT, 1), np.float32)
    shared["onesr"] = np.ones((1, PT), np.float32)
    sel = np.zeros((BC, NPAIR * PT), np.float32)
    for i in range(NPAIR):
        sel[2 * i, i * PT:i * PT + S] = 1.0
        sel[2 * i + 1, i * PT + S:(i + 1) * PT] = 1.0
    shared["sel_all"] = sel
    third = np.zeros((LP, HBC), np.float32)
    third[np.arange(LP), np.arange(LP) // LAST] = 1.0 / LAST
    shared["third"] = third

    in_maps = []
    for c in range(N_CORES):
        sl = slice(c * BC, (c + 1) * BC)
        inp = fi32(inputs[sl])                       # [64, 50]
        idx_cols = np.ascontiguousarray(inp.reshape(NPAIR, PT).T)
        adjT = np.ascontiguousarray(
            np.asarray(adj[sl], dtype=np.float32).transpose(0, 2, 1))
        lc = fi32(last_items[sl])                    # [64, 3]
        last_cols = np.ascontiguousarray(lc.reshape(2, LP).T)
        ai = fi32(adj_items[sl])                     # [64, 36]
        adjit_cols = np.ascontiguousarray(
            ai.reshape(2, LP, NADJ).transpose(1, 0, 2).reshape(LP, 2 * NADJ))
        m = dict(shared)
        m.update({
            "idx_cols": idx_cols,
            "adjT": adjT,
            "last_cols": last_cols,
            "adjit_cols": adjit_cols,
        })
        in_maps.append(m)
    return in_maps


def _assemble(results):
    outs = []
    lasts = []
    for r in results:
        outT = np.asarray(r["outT"])                 # [2, 50, 32, 100]
        outs.append(outT.transpose(2, 0, 1, 3).reshape(BC, S, D))
        lasts.append(np.asarray(r["lasth"]).reshape(BC, D))
    return np.concatenate(outs, 0), np.concatenate(lasts, 0)


def kernel(**inputs):
    nc = _get_program()
    in_maps = _prep_maps(
        inputs["inputs"], inputs["adj"], inputs["last_items"],
        inputs["adj_items"], inputs["embedding"], inputs["s"],
        inputs["W_star_a"], inputs["W_star_q"], inputs["W_star_k"],
        inputs["W_g"], inputs["W_l"], inputs["adj_all"], inputs["num_w"])
    res = run_bass_kernel_spmd(nc, in_maps, list(range(N_CORES)))
    return _assemble(res.results)
